# revision 1
# baseline (speedup 1.0000x reference)
"""ChessGNN (2-layer GCN + global max pool + FC + log_softmax) on 8 Trainium2 cores.

Strategy (edge-parallel, dst-range sharded):
  - Core k owns dst nodes [12500k, 12500(k+1)). Edges are routed to the core
    owning their dst. Within a core, edges are ordered by (src-range, dst-bucket)
    where a bucket is 128 consecutive dst nodes.
  - Per layer: every core computes hs = (h_prev @ W) * dinv for its node slice,
    writes it to a padded [12544, 64] buffer, AllGather -> [100352, 64] table.
  - Messages are fetched with the GPSIMD dma_gather (1024 idxs / instruction,
    int16 idxs => 4 src ranges of 32768 rows).
  - Segment-sum is matmul-based: per 128-edge chunk, a one-hot [128e, 128slot]
    matrix (DVE is_equal vs iota) is multiplied with the 128x32 message block on
    the PE, accumulating per-bucket in PSUM. No scatter primitives (their CCE
    add loses updates on duplicate indices).
  - deg is computed with the same one-hot trick (rhs = ones). dinv = 1/sqrt(deg+1).
  - Head: local max over slice -> PE transpose -> AllReduce(max) -> FC -> log_softmax.
"""
import numpy as np

import concourse.bass as bass
import concourse.bacc as bacc
import concourse.mybir as mybir
import concourse.tile as tile
from concourse.bass_utils import run_bass_kernel_spmd
from concourse.masks import make_identity

N = 100000
NCORES = 8
S = N // NCORES            # 12500 nodes per core
NB = 98                    # buckets of 128 dst nodes (98*128 = 12544)
SP = NB * 128              # padded slice rows
NPAD = NCORES * SP         # padded global rows = 100352
RNG = 32768                # int16 gather range
NRANGES = (NPAD + RNG - 1) // RNG  # 4
PADDLOC = 999.0

LAST_RESULTS = None
RUN_WALL_NS = None


def _prep_core(src_g, dl, ranges_cnt_max=None):
    """Order core edges by (src-range, bucket); return per-(g,b) counts or padded arrays."""
    rg = src_g >> 15
    b = dl >> 7
    order = np.lexsort((b, rg))
    return src_g[order], dl[order], rg[order], b[order]


def kernel(x, edge_index, W1, b1, W2, b2, fcW, fcb):
    global LAST_RESULTS
    x = np.asarray(x, np.float32)
    ei = np.asarray(edge_index)
    src = ei[0].astype(np.int64)
    dst = ei[1].astype(np.int64)

    # ---- host sharding / marshaling ----
    owner = dst // S
    src_gid = (src // S) * SP + (src % S)      # padded global row id of src
    per = []
    for k in range(NCORES):
        m = owner == k
        sg, dl, rg, bb = _prep_core(src_gid[m], (dst[m] - k * S).astype(np.int64))
        per.append((sg, dl, rg, bb))

    # per (g, b) chunk counts = max over cores, rounded to 128
    cnt = np.zeros((NCORES, NRANGES, NB), np.int64)
    for k in range(NCORES):
        sg, dl, rg, bb = per[k]
        np.add.at(cnt[k], (rg, bb), 1)
    chunks = (cnt.max(axis=0) + 127) // 128            # [NRANGES, NB]
    # pad each range's chunk count to a multiple of 8 (1024-idx gather instrs)
    cg = chunks.sum(axis=1)
    extra = (-cg) % 8
    chunks[:, NB - 1] += extra
    cg = chunks.sum(axis=1)                             # chunks per range
    C = int(cg.sum())                                   # total chunks
    NI = [int(c) // 8 for c in cg]                      # gather instrs per range
    NI_tot = sum(NI)

    # build per-core padded idx/dloc arrays in the global chunk grid
    g16_all, dlf_all = [], []
    for k in range(NCORES):
        sg, dl, rg, bb = per[k]
        gidx = np.zeros(C * 128, np.int16)
        dloc = np.full(C * 128, PADDLOC, np.float32)
        pos = 0
        ptr = 0
        for g in range(NRANGES):
            for b in range(NB):
                n = int(cnt[k, g, b])
                cap = int(chunks[g, b]) * 128
                sl = slice(ptr, ptr + n)
                gidx[pos:pos + n] = (sg[sl] - g * RNG).astype(np.int16)
                dloc[pos:pos + n] = (dl[sl] - b * 128).astype(np.float32)
                ptr += n
                pos += cap
        # gather idx slabs: instruction m covers idxs [1024m,1024(m+1)),
        # idx i -> [i%16, i//16] of a [16,64] slab, replicated to 128 partitions.
        slabs = gidx.reshape(NI_tot, 64, 16).transpose(0, 2, 1)      # [NI,16,64]
        g16 = np.tile(slabs.reshape(1, NI_tot * 16, 64)
                      .reshape(NI_tot, 16, 64), (1, 8, 1)).transpose(1, 0, 2) \
                .reshape(128, NI_tot * 64)
        # dloc layout: chunk j, lane p -> [p, j]
        dlf = dloc.reshape(C, 128).T.copy()
        g16_all.append(np.ascontiguousarray(g16))
        dlf_all.append(dlf)

    xT_all = []
    for k in range(NCORES):
        xs = np.zeros((SP, 8), np.float32)
        xs[:S] = x[k * S:(k + 1) * S]
        xT_all.append(np.ascontiguousarray(xs.T))

    iota = np.tile(np.arange(128, dtype=np.float32), (128, 1))
    b1t = np.tile(np.asarray(b1, np.float32)[None, :], (128, 1))
    b2t = np.tile(np.asarray(b2, np.float32)[None, :], (128, 1))
    fcb2 = np.asarray(fcb, np.float32)[None, :]

    # ---- build the SPMD program ----
    nc = bacc.Bacc("TRN2", target_bir_lowering=False, debug=False, num_devices=NCORES)
    dt = mybir.dt
    xT_t = nc.dram_tensor("xT", [8, SP], dt.float32, kind="ExternalInput")
    g16_t = nc.dram_tensor("g16", [128, NI_tot * 64], dt.int16, kind="ExternalInput")
    dlf_t = nc.dram_tensor("dlf", [128, C], dt.float32, kind="ExternalInput")
    iota_t = nc.dram_tensor("iota", [128, 128], dt.float32, kind="ExternalInput")
    W1_t = nc.dram_tensor("W1", [8, 32], dt.float32, kind="ExternalInput")
    W2_t = nc.dram_tensor("W2", [32, 32], dt.float32, kind="ExternalInput")
    b1_t = nc.dram_tensor("b1t", [128, 32], dt.float32, kind="ExternalInput")
    b2_t = nc.dram_tensor("b2t", [128, 32], dt.float32, kind="ExternalInput")
    fcW_t = nc.dram_tensor("fcW", [32, 5], dt.float32, kind="ExternalInput")
    fcb_t = nc.dram_tensor("fcb", [1, 5], dt.float32, kind="ExternalInput")
    out_t = nc.dram_tensor("out", [1, 5], dt.float32, kind="ExternalOutput")

    AF = mybir.ActivationFunctionType
    ALU = mybir.AluOpType
    AX = mybir.AxisListType

    with tile.TileContext(nc) as tc:
        with (
            tc.tile_pool(name="per", bufs=1) as per_p,
            tc.tile_pool(name="gt", bufs=4) as gt_p,
            tc.tile_pool(name="oh", bufs=4) as oh_p,
            tc.tile_pool(name="ps", bufs=2, space="PSUM") as ps_p,
            tc.tile_pool(name="psd", bufs=1, space="PSUM") as psd_p,
            tc.tile_pool(name="dram", bufs=1, space="DRAM") as dram_p,
        ):
            # persistent tiles
            xT = per_p.tile([8, SP], dt.float32)
            G16 = per_p.tile([128, NI_tot * 64], dt.int16)
            DLF = per_p.tile([128, C], dt.float32)
            IO = per_p.tile([128, 128], dt.float32)
            W1s = per_p.tile([8, 32], dt.float32)
            W2s = per_p.tile([32, 32], dt.float32)
            B1 = per_p.tile([128, 32], dt.float32)
            B2 = per_p.tile([128, 32], dt.float32)
            FCW = per_p.tile([32, 5], dt.float32)
            FCB = per_p.tile([1, 5], dt.float32)
            ONES = per_p.tile([128, 1], dt.float32)
            IDN = per_p.tile([128, 128], dt.float32)
            P = per_p.tile([128, NB, 32], dt.float32)    # h_prev @ W (slice)
            H = per_p.tile([128, NB, 32], dt.float32)    # layer output
            HS = per_p.tile([128, NB, 32], dt.float32)   # P * dinv
            ACC = per_p.tile([128, NB, 32], dt.float32)  # aggregated messages
            DEG = per_p.tile([128, NB], dt.float32)
            DINV = per_p.tile([128, NB], dt.float32)

            for t_, s_ in ((xT, xT_t), (G16, g16_t), (DLF, dlf_t), (IO, iota_t),
                           (W1s, W1_t), (W2s, W2_t), (B1, b1_t), (B2, b2_t),
                           (FCW, fcW_t), (FCB, fcb_t)):
                nc.sync.dma_start(t_[:], s_[:, :])
            nc.vector.memset(ONES[:], 1.0)
            make_identity(nc, IDN[:])

            agin1 = dram_p.tile([SP, 64], dt.float32)
            agout1 = dram_p.tile([NPAD, 64], dt.float32)
            agin2 = dram_p.tile([SP, 64], dt.float32)
            agout2 = dram_p.tile([NPAD, 64], dt.float32)
            arin = dram_p.tile([32, 1], dt.float32)
            arout = dram_p.tile([32, 1], dt.float32)

            # ---- P1 = x @ W1 (per 128-node tile) ----
            for t in range(NB):
                psm = ps_p.tile([128, 32], dt.float32, tag="pmm")
                nc.tensor.matmul(psm[:], lhsT=xT[:, t * 128:(t + 1) * 128],
                                 rhs=W1s[:], start=True, stop=True)
                nc.scalar.copy(P[:, t, :], psm[:])

            # ---- deg via one-hot matmuls (rhs = ones) ----
            nc.vector.memset(DEG[:], 0.0)
            jg = 0
            psd = None
            for g in range(NRANGES):
                flat = []
                for b in range(NB):
                    nch = int(chunks[g, b])
                    for c in range(nch):
                        flat.append((b, c == 0, c == nch - 1))
                for m in range(len(flat) // 8):
                    oh = oh_p.tile([128, 8, 128], dt.float32, tag="oha")
                    j0 = jg + m * 8
                    nc.vector.tensor_tensor(
                        out=oh[:],
                        in0=DLF[:, j0:j0 + 8].rearrange("p (c o) -> p c o", o=1)
                            .to_broadcast([128, 8, 128]),
                        in1=IO[:].rearrange("p (o s) -> p o s", o=1)
                            .to_broadcast([128, 8, 128]),
                        op=ALU.is_equal)
                    for s in range(8):
                        b, first, last = flat[m * 8 + s]
                        if first:
                            psd = psd_p.tile([128, 1], dt.float32, tag="pdeg")
                        nc.tensor.matmul(psd[:], lhsT=oh[:, s, :], rhs=ONES[:],
                                         start=first, stop=last)
                        if last:
                            nc.vector.tensor_add(DEG[:, b:b + 1], DEG[:, b:b + 1], psd[:])
                jg += len(flat)
            # dinv = 1/sqrt(deg + 1)
            SQ = per_p.tile([128, NB], dt.float32)
            nc.scalar.activation(SQ[:], DEG[:], AF.Sqrt, bias=1.0)
            nc.vector.reciprocal(DINV[:], SQ[:])

            dinv_b = DINV[:].rearrange("p (b o) -> p b o", o=1).to_broadcast([128, NB, 32])

            def aggregate(agout, acc):
                """gather + one-hot matmul segment sum over all chunks."""
                nc.vector.memset(acc[:], 0.0)
                jg2 = 0
                mi = 0
                for g in range(NRANGES):
                    r0 = g * RNG
                    r1 = min((g + 1) * RNG, NPAD)
                    src_ap = agout[r0:r1, :]
                    # per-range schedule: (bucket, count) pairs
                    sched = [(b, int(chunks[g, b])) for b in range(NB) if chunks[g, b] > 0]
                    flat = []
                    for b, nch in sched:
                        for c in range(nch):
                            flat.append((b, c == 0, c == nch - 1))
                    ntiles = len(flat) // 8
                    for m in range(ntiles):
                        gt = gt_p.tile([128, 8, 64], dt.float32, tag="gt")
                        nc.gpsimd.dma_gather(gt[:], src_ap, G16[:, mi * 64:(mi + 1) * 64],
                                             1024, 1024, 64)
                        oh = oh_p.tile([128, 8, 128], dt.float32, tag="oha")
                        j0 = jg2 + m * 8
                        nc.vector.tensor_tensor(
                            out=oh[:],
                            in0=DLF[:, j0:j0 + 8].rearrange("p (c o) -> p c o", o=1)
                                .to_broadcast([128, 8, 128]),
                            in1=IO[:].rearrange("p (o s) -> p o s", o=1)
                                .to_broadcast([128, 8, 128]),
                            op=ALU.is_equal)
                        for s in range(8):
                            b, first, last = flat[m * 8 + s]
                            if first:
                                psm = ps_p.tile([128, 32], dt.float32, tag="pagg")
                            nc.tensor.matmul(psm[:], lhsT=oh[:, s, :], rhs=gt[:, s, 0:32],
                                             start=first, stop=last)
                            if last:
                                nc.vector.tensor_add(acc[:, b, :], acc[:, b, :], psm[:])
                        mi += 1
                    jg2 += len(flat)

            def combine(acc, Pt, Bt, h):
                """h = relu(dinv*(acc + dinv*P) + b)"""
                T1 = per_p.tile([128, NB, 32], dt.float32, tag="t1")
                nc.vector.tensor_mul(T1[:], Pt[:], dinv_b)
                nc.vector.tensor_add(T1[:], T1[:], acc[:])
                nc.vector.tensor_mul(T1[:], T1[:], dinv_b)
                nc.vector.tensor_add(
                    T1[:], T1[:],
                    Bt[:].rearrange("p (o f) -> p o f", o=1).to_broadcast([128, NB, 32]))
                nc.scalar.activation(h[:], T1[:], AF.Relu)

            # ---- layer 1 ----
            nc.vector.tensor_mul(HS[:], P[:], dinv_b)
            nc.sync.dma_start(
                agin1[:, :].rearrange("(a p) b -> p a b", p=128)[:, :, 0:32], HS[:])
            nc.gpsimd.collective_compute(
                "AllGather", ALU.bypass, replica_groups=[list(range(NCORES))],
                ins=[agin1.opt()], outs=[agout1.opt()])
            aggregate(agout1, ACC)
            combine(ACC, P, B1, H)

            # ---- P2 = h1 @ W2 via per-tile transpose ----
            for t in range(NB):
                pst = psd_p.tile([32, 128], dt.float32, tag="ptr")
                nc.tensor.transpose(out=pst[:], in_=H[:, t, :], identity=IDN[:])
                h1t = gt_p.tile([32, 128], dt.float32, tag="h1t")
                nc.scalar.copy(h1t[:], pst[:])
                psm = ps_p.tile([128, 32], dt.float32, tag="pmm")
                nc.tensor.matmul(psm[:], lhsT=h1t[:], rhs=W2s[:], start=True, stop=True)
                nc.scalar.copy(P[:, t, :], psm[:])

            # ---- layer 2 ----
            nc.vector.tensor_mul(HS[:], P[:], dinv_b)
            nc.sync.dma_start(
                agin2[:, :].rearrange("(a p) b -> p a b", p=128)[:, :, 0:32], HS[:])
            nc.gpsimd.collective_compute(
                "AllGather", ALU.bypass, replica_groups=[list(range(NCORES))],
                ins=[agin2.opt()], outs=[agout2.opt()])
            aggregate(agout2, ACC)
            combine(ACC, P, B2, H)

            # ---- head: global max pool + FC + log_softmax ----
            GMAX = per_p.tile([128, 32], dt.float32)
            nc.vector.tensor_copy(GMAX[:], H[:, 0, :])
            for t in range(1, NB):
                nc.vector.tensor_tensor(GMAX[:], GMAX[:], H[:, t, :], op=ALU.max)
            psg = psd_p.tile([32, 128], dt.float32, tag="ptr")
            nc.tensor.transpose(out=psg[:], in_=GMAX[:], identity=IDN[:])
            GT = per_p.tile([32, 128], dt.float32)
            nc.scalar.copy(GT[:], psg[:])
            GV = per_p.tile([32, 1], dt.float32)
            nc.vector.reduce_max(GV[:], GT[:], axis=AX.X)
            nc.sync.dma_start(arin[:, :], GV[:])
            nc.gpsimd.collective_compute(
                "AllReduce", ALU.max, replica_groups=[list(range(NCORES))],
                ins=[arin.opt()], outs=[arout.opt()])
            GAR = per_p.tile([32, 1], dt.float32)
            nc.sync.dma_start(GAR[:], arout[:, :])
            psl = psd_p.tile([1, 5], dt.float32, tag="plg")
            nc.tensor.matmul(psl[:], lhsT=GAR[:], rhs=FCW[:], start=True, stop=True)
            LG = per_p.tile([1, 5], dt.float32)
            nc.vector.tensor_add(LG[:], psl[:], FCB[:])
            MX = per_p.tile([1, 1], dt.float32)
            nc.vector.reduce_max(MX[:], LG[:], axis=AX.X)
            nc.vector.tensor_tensor(LG[:], LG[:], MX[:].to_broadcast([1, 5]),
                                    op=ALU.subtract)
            EX = per_p.tile([1, 5], dt.float32)
            nc.scalar.activation(EX[:], LG[:], AF.Exp)
            SM = per_p.tile([1, 1], dt.float32)
            nc.vector.reduce_sum(SM[:], EX[:], axis=AX.X)
            LS = per_p.tile([1, 1], dt.float32)
            nc.scalar.activation(LS[:], SM[:], AF.Ln)
            nc.vector.tensor_tensor(LG[:], LG[:], LS[:].to_broadcast([1, 5]),
                                    op=ALU.subtract)
            nc.sync.dma_start(out_t[:, :], LG[:])

    nc.compile()

    in_maps = []
    for k in range(NCORES):
        in_maps.append({
            "xT": xT_all[k], "g16": g16_all[k], "dlf": dlf_all[k], "iota": iota,
            "W1": np.asarray(W1, np.float32), "W2": np.asarray(W2, np.float32),
            "b1t": b1t, "b2t": b2t, "fcW": np.asarray(fcW, np.float32), "fcb": fcb2,
        })
    import os, time as _time
    global RUN_WALL_NS
    trace = os.environ.get("GNN_TRACE", "0") == "1"
    _t0 = _time.time()
    res = run_bass_kernel_spmd(nc, in_maps, core_ids=list(range(NCORES)), trace=trace)
    RUN_WALL_NS = int((_time.time() - _t0) * 1e9)
    LAST_RESULTS = res
    return res.results[0]["out"].astype(np.float32)



# revision 3
# speedup vs baseline: 25.1080x; 25.1080x over previous
"""ChessGNN (2-layer GCN + global max pool + FC + log_softmax) on 8 Trainium2 cores.

Strategy (edge-parallel, dst-range sharded):
  - Core k owns dst nodes [12500k, 12500(k+1)). Edges are routed to the core
    owning their dst. Within a core, edges are ordered by (src-range, dst-bucket)
    where a bucket is 128 consecutive dst nodes.
  - Per layer: every core computes hs = (h_prev @ W) * dinv for its node slice,
    writes it to a padded [12544, 64] buffer, AllGather -> [100352, 64] table.
  - Messages are fetched with the GPSIMD dma_gather (1024 idxs / instruction,
    int16 idxs => 4 src ranges of 32768 rows).
  - Segment-sum is matmul-based: per 128-edge chunk, a one-hot [128e, 128slot]
    matrix (DVE is_equal vs iota) is multiplied with the 128x32 message block on
    the PE, accumulating per-bucket in PSUM. No scatter primitives (their CCE
    add loses updates on duplicate indices).
  - deg is computed with the same one-hot trick (rhs = ones). dinv = 1/sqrt(deg+1).
  - Head: local max over slice -> PE transpose -> AllReduce(max) -> FC -> log_softmax.
"""
import numpy as np

import concourse.bass as bass
import concourse.bacc as bacc
import concourse.mybir as mybir
import concourse.tile as tile
from concourse.bass_utils import run_bass_kernel_spmd
from concourse.masks import make_identity

N = 100000
NCORES = 8
S = N // NCORES            # 12500 nodes per core
NB = 98                    # buckets of 128 dst nodes (98*128 = 12544)
SP = NB * 128              # padded slice rows
NPAD = NCORES * SP         # padded global rows = 100352
RNG = 32768                # int16 gather range
NRANGES = (NPAD + RNG - 1) // RNG  # 4
PADDLOC = 999.0

LAST_RESULTS = None
RUN_WALL_NS = None


def _prep_core(src_g, dl, ranges_cnt_max=None):
    """Order core edges by (src-range, bucket); return per-(g,b) counts or padded arrays."""
    rg = src_g >> 15
    b = dl >> 7
    order = np.lexsort((b, rg))
    return src_g[order], dl[order], rg[order], b[order]


def build(x, edge_index, W1, b1, W2, b2, fcW, fcb):
    x = np.asarray(x, np.float32)
    ei = np.asarray(edge_index)
    src = ei[0].astype(np.int64)
    dst = ei[1].astype(np.int64)

    # ---- host sharding / marshaling ----
    owner = dst // S
    src_gid = (src // S) * SP + (src % S)      # padded global row id of src
    per = []
    for k in range(NCORES):
        m = owner == k
        sg, dl, rg, bb = _prep_core(src_gid[m], (dst[m] - k * S).astype(np.int64))
        per.append((sg, dl, rg, bb))

    # per (g, b) chunk counts = max over cores, rounded to 128
    cnt = np.zeros((NCORES, NRANGES, NB), np.int64)
    for k in range(NCORES):
        sg, dl, rg, bb = per[k]
        np.add.at(cnt[k], (rg, bb), 1)
    chunks = (cnt.max(axis=0) + 127) // 128            # [NRANGES, NB]
    # pad each range's chunk count to a multiple of 8 (1024-idx gather instrs)
    cg = chunks.sum(axis=1)
    extra = (-cg) % 8
    chunks[:, NB - 1] += extra
    cg = chunks.sum(axis=1)                             # chunks per range
    C = int(cg.sum())                                   # total chunks
    NI = [int(c) // 8 for c in cg]                      # gather instrs per range
    NI_tot = sum(NI)

    # build per-core padded idx/dloc arrays in the global chunk grid
    g16_all, dlf_all = [], []
    for k in range(NCORES):
        sg, dl, rg, bb = per[k]
        gidx = np.zeros(C * 128, np.int16)
        dloc = np.full(C * 128, PADDLOC, np.float32)
        pos = 0
        ptr = 0
        for g in range(NRANGES):
            for b in range(NB):
                n = int(cnt[k, g, b])
                cap = int(chunks[g, b]) * 128
                sl = slice(ptr, ptr + n)
                gidx[pos:pos + n] = (sg[sl] - g * RNG).astype(np.int16)
                dloc[pos:pos + n] = (dl[sl] - b * 128).astype(np.float32)
                ptr += n
                pos += cap
        # gather idx slabs: instruction m covers idxs [1024m,1024(m+1)),
        # idx i -> [i%16, i//16] of a [16,64] slab, replicated to 128 partitions.
        slabs = gidx.reshape(NI_tot, 64, 16).transpose(0, 2, 1)      # [NI,16,64]
        g16 = np.tile(slabs.reshape(1, NI_tot * 16, 64)
                      .reshape(NI_tot, 16, 64), (1, 8, 1)).transpose(1, 0, 2) \
                .reshape(128, NI_tot * 64)
        # dloc layout: chunk j, lane p -> [p, j]
        dlf = dloc.reshape(C, 128).T.copy()
        g16_all.append(np.ascontiguousarray(g16))
        dlf_all.append(dlf)

    xT_all = []
    for k in range(NCORES):
        xs = np.zeros((SP, 8), np.float32)
        xs[:S] = x[k * S:(k + 1) * S]
        xT_all.append(np.ascontiguousarray(xs.T))

    iota = np.tile(np.arange(128, dtype=np.float32), (128, 1))
    b1t = np.tile(np.asarray(b1, np.float32)[None, :], (128, 1))
    b2t = np.tile(np.asarray(b2, np.float32)[None, :], (128, 1))
    fcb2 = np.asarray(fcb, np.float32)[None, :]

    # ---- build the SPMD program ----
    nc = bacc.Bacc("TRN2", target_bir_lowering=False, debug=False, num_devices=NCORES)
    dt = mybir.dt
    xT_t = nc.dram_tensor("xT", [8, SP], dt.float32, kind="ExternalInput")
    g16_t = nc.dram_tensor("g16", [128, NI_tot * 64], dt.int16, kind="ExternalInput")
    dlf_t = nc.dram_tensor("dlf", [128, C], dt.float32, kind="ExternalInput")
    iota_t = nc.dram_tensor("iota", [128, 128], dt.float32, kind="ExternalInput")
    W1_t = nc.dram_tensor("W1", [8, 32], dt.float32, kind="ExternalInput")
    W2_t = nc.dram_tensor("W2", [32, 32], dt.float32, kind="ExternalInput")
    b1_t = nc.dram_tensor("b1t", [128, 32], dt.float32, kind="ExternalInput")
    b2_t = nc.dram_tensor("b2t", [128, 32], dt.float32, kind="ExternalInput")
    fcW_t = nc.dram_tensor("fcW", [32, 5], dt.float32, kind="ExternalInput")
    fcb_t = nc.dram_tensor("fcb", [1, 5], dt.float32, kind="ExternalInput")
    out_t = nc.dram_tensor("out", [1, 5], dt.float32, kind="ExternalOutput")

    AF = mybir.ActivationFunctionType
    ALU = mybir.AluOpType
    AX = mybir.AxisListType

    with tile.TileContext(nc) as tc:
        with (
            tc.tile_pool(name="per", bufs=1) as per_p,
            tc.tile_pool(name="gt", bufs=4) as gt_p,
            tc.tile_pool(name="oh", bufs=4) as oh_p,
            tc.tile_pool(name="ps", bufs=2, space="PSUM") as ps_p,
            tc.tile_pool(name="psd", bufs=1, space="PSUM") as psd_p,
            tc.tile_pool(name="dram", bufs=1, space="DRAM") as dram_p,
        ):
            # persistent tiles
            xT = per_p.tile([8, SP], dt.float32)
            G16 = per_p.tile([128, NI_tot * 64], dt.int16)
            DLF = per_p.tile([128, C], dt.float32)
            IO = per_p.tile([128, 128], dt.float32)
            W1s = per_p.tile([8, 32], dt.float32)
            W2s = per_p.tile([32, 32], dt.float32)
            B1 = per_p.tile([128, 32], dt.float32)
            B2 = per_p.tile([128, 32], dt.float32)
            FCW = per_p.tile([32, 5], dt.float32)
            FCB = per_p.tile([1, 5], dt.float32)
            ONES = per_p.tile([128, 1], dt.float32)
            IDN = per_p.tile([128, 128], dt.float32)
            P = per_p.tile([128, NB, 32], dt.float32)    # h_prev @ W (slice)
            H = per_p.tile([128, NB, 32], dt.float32)    # layer output
            HS = per_p.tile([128, NB, 32], dt.float32)   # P * dinv
            ACC = per_p.tile([128, NB, 32], dt.float32)  # aggregated messages
            DEG = per_p.tile([128, NB], dt.float32)
            DINV = per_p.tile([128, NB], dt.float32)

            for t_, s_ in ((xT, xT_t), (G16, g16_t), (DLF, dlf_t), (IO, iota_t),
                           (W1s, W1_t), (W2s, W2_t), (B1, b1_t), (B2, b2_t),
                           (FCW, fcW_t), (FCB, fcb_t)):
                nc.sync.dma_start(t_[:], s_[:, :])
            nc.vector.memset(ONES[:], 1.0)
            make_identity(nc, IDN[:])

            agin1 = dram_p.tile([SP, 64], dt.float32)
            agout1 = dram_p.tile([NPAD, 64], dt.float32)
            agin2 = dram_p.tile([SP, 64], dt.float32)
            agout2 = dram_p.tile([NPAD, 64], dt.float32)
            arin = dram_p.tile([32, 1], dt.float32)
            arout = dram_p.tile([32, 1], dt.float32)

            # ---- P1 = x @ W1 (per 128-node tile) ----
            for t in range(NB):
                psm = ps_p.tile([128, 32], dt.float32, tag="pmm")
                nc.tensor.matmul(psm[:], lhsT=xT[:, t * 128:(t + 1) * 128],
                                 rhs=W1s[:], start=True, stop=True)
                nc.scalar.copy(P[:, t, :], psm[:])

            # ---- deg via one-hot matmuls (rhs = ones) ----
            nc.vector.memset(DEG[:], 0.0)
            jg = 0
            psd = None
            for g in range(NRANGES):
                flat = []
                for b in range(NB):
                    nch = int(chunks[g, b])
                    for c in range(nch):
                        flat.append((b, c == 0, c == nch - 1))
                for m in range(len(flat) // 8):
                    oh = oh_p.tile([128, 8, 128], dt.float32, tag="oha")
                    j0 = jg + m * 8
                    nc.vector.tensor_tensor(
                        out=oh[:],
                        in0=DLF[:, j0:j0 + 8].rearrange("p (c o) -> p c o", o=1)
                            .to_broadcast([128, 8, 128]),
                        in1=IO[:].rearrange("p (o s) -> p o s", o=1)
                            .to_broadcast([128, 8, 128]),
                        op=ALU.is_equal)
                    for s in range(8):
                        b, first, last = flat[m * 8 + s]
                        if first:
                            psd = psd_p.tile([128, 1], dt.float32, tag="pdeg")
                        nc.tensor.matmul(psd[:], lhsT=oh[:, s, :], rhs=ONES[:],
                                         start=first, stop=last)
                        if last:
                            nc.vector.tensor_add(DEG[:, b:b + 1], DEG[:, b:b + 1], psd[:])
                jg += len(flat)
            # dinv = 1/sqrt(deg + 1)
            SQ = per_p.tile([128, NB], dt.float32)
            nc.scalar.activation(SQ[:], DEG[:], AF.Sqrt, bias=1.0)
            nc.vector.reciprocal(DINV[:], SQ[:])

            dinv_b = DINV[:].rearrange("p (b o) -> p b o", o=1).to_broadcast([128, NB, 32])

            def aggregate(agout, acc):
                """gather + one-hot matmul segment sum over all chunks."""
                nc.vector.memset(acc[:], 0.0)
                jg2 = 0
                mi = 0
                for g in range(NRANGES):
                    r0 = g * RNG
                    r1 = min((g + 1) * RNG, NPAD)
                    src_ap = agout[r0:r1, :]
                    # per-range schedule: (bucket, count) pairs
                    sched = [(b, int(chunks[g, b])) for b in range(NB) if chunks[g, b] > 0]
                    flat = []
                    for b, nch in sched:
                        for c in range(nch):
                            flat.append((b, c == 0, c == nch - 1))
                    ntiles = len(flat) // 8
                    for m in range(ntiles):
                        gt = gt_p.tile([128, 8, 64], dt.float32, tag="gt")
                        nc.gpsimd.dma_gather(gt[:], src_ap, G16[:, mi * 64:(mi + 1) * 64],
                                             1024, 1024, 64)
                        oh = oh_p.tile([128, 8, 128], dt.float32, tag="oha")
                        j0 = jg2 + m * 8
                        nc.vector.tensor_tensor(
                            out=oh[:],
                            in0=DLF[:, j0:j0 + 8].rearrange("p (c o) -> p c o", o=1)
                                .to_broadcast([128, 8, 128]),
                            in1=IO[:].rearrange("p (o s) -> p o s", o=1)
                                .to_broadcast([128, 8, 128]),
                            op=ALU.is_equal)
                        for s in range(8):
                            b, first, last = flat[m * 8 + s]
                            if first:
                                psm = ps_p.tile([128, 32], dt.float32, tag="pagg")
                            nc.tensor.matmul(psm[:], lhsT=oh[:, s, :], rhs=gt[:, s, 0:32],
                                             start=first, stop=last)
                            if last:
                                nc.vector.tensor_add(acc[:, b, :], acc[:, b, :], psm[:])
                        mi += 1
                    jg2 += len(flat)

            def combine(acc, Pt, Bt, h):
                """h = relu(dinv*(acc + dinv*P) + b)"""
                T1 = per_p.tile([128, NB, 32], dt.float32, tag="t1")
                nc.vector.tensor_mul(T1[:], Pt[:], dinv_b)
                nc.vector.tensor_add(T1[:], T1[:], acc[:])
                nc.vector.tensor_mul(T1[:], T1[:], dinv_b)
                nc.vector.tensor_add(
                    T1[:], T1[:],
                    Bt[:].rearrange("p (o f) -> p o f", o=1).to_broadcast([128, NB, 32]))
                nc.scalar.activation(h[:], T1[:], AF.Relu)

            # ---- layer 1 ----
            nc.vector.tensor_mul(HS[:], P[:], dinv_b)
            nc.sync.dma_start(
                agin1[:, :].rearrange("(a p) b -> p a b", p=128)[:, :, 0:32], HS[:])
            nc.gpsimd.collective_compute(
                "AllGather", ALU.bypass, replica_groups=[list(range(NCORES))],
                ins=[agin1.opt()], outs=[agout1.opt()])
            aggregate(agout1, ACC)
            combine(ACC, P, B1, H)

            # ---- P2 = h1 @ W2 via per-tile transpose ----
            for t in range(NB):
                pst = psd_p.tile([32, 128], dt.float32, tag="ptr")
                nc.tensor.transpose(out=pst[:], in_=H[:, t, :], identity=IDN[:])
                h1t = gt_p.tile([32, 128], dt.float32, tag="h1t")
                nc.scalar.copy(h1t[:], pst[:])
                psm = ps_p.tile([128, 32], dt.float32, tag="pmm")
                nc.tensor.matmul(psm[:], lhsT=h1t[:], rhs=W2s[:], start=True, stop=True)
                nc.scalar.copy(P[:, t, :], psm[:])

            # ---- layer 2 ----
            nc.vector.tensor_mul(HS[:], P[:], dinv_b)
            nc.sync.dma_start(
                agin2[:, :].rearrange("(a p) b -> p a b", p=128)[:, :, 0:32], HS[:])
            nc.gpsimd.collective_compute(
                "AllGather", ALU.bypass, replica_groups=[list(range(NCORES))],
                ins=[agin2.opt()], outs=[agout2.opt()])
            aggregate(agout2, ACC)
            combine(ACC, P, B2, H)

            # ---- head: global max pool + FC + log_softmax ----
            GMAX = per_p.tile([128, 32], dt.float32)
            nc.vector.tensor_copy(GMAX[:], H[:, 0, :])
            for t in range(1, NB):
                nc.vector.tensor_tensor(GMAX[:], GMAX[:], H[:, t, :], op=ALU.max)
            psg = psd_p.tile([32, 128], dt.float32, tag="ptr")
            nc.tensor.transpose(out=psg[:], in_=GMAX[:], identity=IDN[:])
            GT = per_p.tile([32, 128], dt.float32)
            nc.scalar.copy(GT[:], psg[:])
            GV = per_p.tile([32, 1], dt.float32)
            nc.vector.reduce_max(GV[:], GT[:], axis=AX.X)
            nc.sync.dma_start(arin[:, :], GV[:])
            nc.gpsimd.collective_compute(
                "AllReduce", ALU.max, replica_groups=[list(range(NCORES))],
                ins=[arin.opt()], outs=[arout.opt()])
            GAR = per_p.tile([32, 1], dt.float32)
            nc.sync.dma_start(GAR[:], arout[:, :])
            psl = psd_p.tile([1, 5], dt.float32, tag="plg")
            nc.tensor.matmul(psl[:], lhsT=GAR[:], rhs=FCW[:], start=True, stop=True)
            LG = per_p.tile([1, 5], dt.float32)
            nc.vector.tensor_add(LG[:], psl[:], FCB[:])
            MX = per_p.tile([1, 1], dt.float32)
            nc.vector.reduce_max(MX[:], LG[:], axis=AX.X)
            nc.vector.tensor_tensor(LG[:], LG[:], MX[:].to_broadcast([1, 5]),
                                    op=ALU.subtract)
            EX = per_p.tile([1, 5], dt.float32)
            nc.scalar.activation(EX[:], LG[:], AF.Exp)
            SM = per_p.tile([1, 1], dt.float32)
            nc.vector.reduce_sum(SM[:], EX[:], axis=AX.X)
            LS = per_p.tile([1, 1], dt.float32)
            nc.scalar.activation(LS[:], SM[:], AF.Ln)
            nc.vector.tensor_tensor(LG[:], LG[:], LS[:].to_broadcast([1, 5]),
                                    op=ALU.subtract)
            nc.sync.dma_start(out_t[:, :], LG[:])

    nc.compile()

    in_maps = []
    for k in range(NCORES):
        in_maps.append({
            "xT": xT_all[k], "g16": g16_all[k], "dlf": dlf_all[k], "iota": iota,
            "W1": np.asarray(W1, np.float32), "W2": np.asarray(W2, np.float32),
            "b1t": b1t, "b2t": b2t, "fcW": np.asarray(fcW, np.float32), "fcb": fcb2,
        })
    return nc, in_maps


def kernel(x, edge_index, W1, b1, W2, b2, fcW, fcb):
    global LAST_RESULTS, RUN_WALL_NS
    nc, in_maps = build(x, edge_index, W1, b1, W2, b2, fcW, fcb)
    import os, time as _time
    trace = os.environ.get("GNN_TRACE", "0") == "1"
    _t0 = _time.time()
    res = run_bass_kernel_spmd(nc, in_maps, core_ids=list(range(NCORES)), trace=trace)
    RUN_WALL_NS = int((_time.time() - _t0) * 1e9)
    LAST_RESULTS = res
    return res.results[0]["out"].astype(np.float32)

